# revision 14
# baseline (speedup 1.0000x reference)
"""Trainium2 Bass kernel for BayesLinearEMP (moe_routing).

out[b] = weights[mode_idx[b]] @ x[b] + biases[mode_idx[b]]
  x: [128, 2048] f32, weights: [20, 2048, 2048] f32, biases: [20, 2048] f32,
  mode_idx: [128] int

Strategy (8 NeuronCores):
  - Split the output dim O=2048 into 8 slices of 256, one per core.  Every
    core reads all 20 modes' weights for its O-slice - perfectly balanced
    regardless of the mode distribution, and total weight traffic is
    read-once (the memory-roofline minimum).
  - On the host, sort samples by mode.  Per mode m with count c_m the core
    computes a [c_m, 256] tile as 16 K-chunk matmuls (K=128, N=256) with the
    x chunk as the stationary operand, accumulated in one PSUM group;
    per-mode counts are compile-time constants (program cached per
    counts-tuple).
  - The problem is DMA-bound (weights dominate traffic; streams measured at
    ~340 GB/s per core vs the ~358 GB/s HBM-per-NC limit), so the weight
    encoding is chosen for minimum bytes within the 2e-2 error budget:
    NF8 of the 16 K-chunks are stored as e4m3 fp8 (x1 byte), the rest as
    bf16 (x2 bytes).  The fp8 chunks' 2^12 quantization scale is folded
    into their x chunks (exact power-of-2) so every chunk accumulates into
    the same PSUM group with no extra instructions.
      NF8=0:  21 MB/core,   measured rel err 2.06e-3, ~86 us
      NF8=8:  16.1 MB/core, measured rel err 1.6543e-2, ~69 us
      NF8=9:  15.4 MB/core, measured rel err 1.7566e-2, ~68 us  (default)
    HW reproduces the numpy quantization sim to ~1e-4 relative; the error
    is a norm over 262k outputs, so it concentrates hard even if the
    harness regenerates inputs - no tail risk at 12% margin.
  - DMA plan: x halves lead the two HWDGE rings, then per mode (processed
    largest-count first) the fp8 tile rides one ring and the bf16 tile the
    other, alternating per mode; 12-deep weight pool keeps ~5 modes of
    prefetch in flight so the stream never starves; the last two modes'
    bf16 tiles are split in half so the tail compute overlaps the final
    bytes.  SWDGE (gpsimd) is avoided: its end-of-kernel Q7 drain costs
    ~4 us.
  - The bias is folded into the PSUM accumulation with a K=1 ones-matmul
    against the bf16 bias row; per-mode results are DVE-copied to SBUF and
    written back per mode so output writes overlap the stream.
"""

import os
import sys

for _p in ("/opt/trn_rl_repo", "/root/.axon_site/_ro/trn_rl_repo"):
    if _p not in sys.path:
        sys.path.append(_p)

import numpy as np
import ml_dtypes

BF16 = ml_dtypes.bfloat16
F8 = ml_dtypes.float8_e4m3

B, I, O, M = 128, 2048, 2048, 20
NCORES = 8
OC = O // NCORES          # 256 output cols per core
KC = I // 128             # 16 contraction chunks

NF8 = int(os.environ.get("BASS_NF8", "9"))   # fp8 chunks per 16
F8_SCALE = 4096.0                            # 2^12, folded into x chunks

_CACHE: dict = {}
LAST_EXEC_TIME_NS = None


def _install_ntff_shim():
    """antenv.axon_hooks is absent in this image; recreate it so the
    trace=True path of run_bass_kernel_spmd can reach NTFF profiling."""
    import types
    import antenv

    if getattr(antenv, "axon_hooks", None) is not None:
        return
    hooks_mod = types.ModuleType("antenv.axon_hooks")
    _hook = [None]
    hooks_mod.set_axon_ntff_profile_hook = lambda h: _hook.__setitem__(0, h)
    hooks_mod.get_axon_ntff_profile_hook = lambda: _hook[0]
    sys.modules["antenv.axon_hooks"] = hooks_mod
    antenv.axon_hooks = hooks_mod
    try:
        from trn_agent_boot.trn_boot import _ntff_profile_via_ctypes

        hooks_mod.set_axon_ntff_profile_hook(
            _ntff_profile_via_ctypes("/opt/axon/libaxon_pjrt.so")
        )
    except Exception:
        pass
    import concourse.bass_utils as bass_utils

    bass_utils.upload_artifacts = lambda tmpdir: "local://" + tmpdir


def _build(counts: tuple, nf8: int):
    import concourse.bass as bass
    import concourse.tile as tile
    from concourse import bacc, mybir

    offs = np.concatenate([[0], np.cumsum(counts)]).astype(int)

    nc = bacc.Bacc("TRN2", target_bir_lowering=False, debug=False, num_devices=NCORES)
    bf = mybir.dt.bfloat16
    f8 = mybir.dt.float8e4
    f32 = mybir.dt.float32

    nbf = KC - nf8
    if nf8:
        w8_d = nc.dram_tensor("w8", [M, 128, nf8 * OC], f8, kind="ExternalInput").ap()
    if nbf:
        wb_d = nc.dram_tensor("wb", [M, 128, nbf * OC], bf, kind="ExternalInput").ap()
    x_d = nc.dram_tensor("x", [128, KC, 128], bf, kind="ExternalInput").ap()
    b_d = nc.dram_tensor("b", [1, M * OC], bf, kind="ExternalInput").ap()
    out_d = nc.dram_tensor("out", [B, OC], f32, kind="ExternalOutput").ap()

    # process modes largest-count first: big output tiles drain early and
    # the tail mode is small
    morder = [m for m in np.argsort(-np.asarray(counts), kind="stable") if counts[m]]

    with tile.TileContext(nc) as tc:
        with (
            tc.tile_pool(name="w", bufs=14) as wpool,
            tc.tile_pool(name="x", bufs=1) as xpool,
            tc.tile_pool(name="consts", bufs=1) as cpool,
            tc.tile_pool(name="o", bufs=4) as opool,
            tc.tile_pool(name="ps", bufs=6, space=bass.MemorySpace.PSUM) as pspool,
        ):
            # x halves lead each HWDGE ring (0.25 MB ahead of mode 0's W);
            # SWDGE stays unused - its end-of-kernel Q7 drain costs ~4 us
            xa = xpool.tile([128, KC // 2, 128], bf, tag="xa")
            nc.sync.dma_start(xa[:], x_d[:, 0 : KC // 2])
            xb = xpool.tile([128, KC // 2, 128], bf, tag="xb")
            nc.scalar.dma_start(xb[:], x_d[:, KC // 2 :])
            x_tiles = (xa, xb)
            bt = cpool.tile([1, M * OC], bf)
            nc.scalar.dma_start(bt[:], b_d[:])
            ones = cpool.tile([1, 128], bf)
            nc.vector.memset(ones[:], 1.0)

            for mi_pos, m in enumerate(morder):
                cm = int(counts[m])
                o0 = int(offs[m])
                # alternate rings per mode to balance the two HWDGE queues
                ring_a = nc.sync if (mi_pos % 2) else nc.scalar
                ring_b = nc.scalar if (mi_pos % 2) else nc.sync
                # split the tail modes' bf16 tile so their compute pipelines
                # with the last bytes of the stream
                split_wb = mi_pos >= len(morder) - 2 and nbf >= 2
                if nf8:
                    w8t = wpool.tile([128, nf8 * OC], f8, tag="w8")
                    ring_a.dma_start(w8t[:], w8_d[m])
                wb_tiles = []
                if nbf:
                    wring = ring_b if nf8 else ring_a
                    if split_wb:
                        h = nbf // 2
                        wb0 = wpool.tile([128, h * OC], bf, tag="wb0", bufs=2)
                        wring.dma_start(wb0[:], wb_d[m, :, 0 : h * OC])
                        wb1 = wpool.tile([128, (nbf - h) * OC], bf, tag="wb1", bufs=2)
                        (ring_a if nf8 else ring_b).dma_start(
                            wb1[:], wb_d[m, :, h * OC :]
                        )
                        wb_tiles = [(wb0, 0), (wb1, h)]
                    else:
                        wbt = wpool.tile([128, nbf * OC], bf, tag="wb")
                        wring.dma_start(wbt[:], wb_d[m])
                        wb_tiles = [(wbt, 0)]

                ps = pspool.tile([128, OC], f32, tag="ps")
                for k in range(KC):
                    if k < nf8:
                        w_k = w8t[:, k * OC : (k + 1) * OC]
                    else:
                        kb = k - nf8
                        wt, k0 = wb_tiles[-1] if (wb_tiles[-1][1] <= kb) else wb_tiles[0]
                        w_k = wt[:, (kb - k0) * OC : (kb - k0 + 1) * OC]
                    x_k = x_tiles[k // (KC // 2)][:, k % (KC // 2), o0 : o0 + cm]
                    nc.tensor.matmul(
                        ps[0:cm, :], x_k, w_k, start=(k == 0), stop=False
                    )
                # bias: ones[1,cm].T @ b_row
                nc.tensor.matmul(
                    ps[0:cm, :],
                    ones[:, 0:cm],
                    bt[:, m * OC : (m + 1) * OC],
                    start=False,
                    stop=True,
                )
                ot = opool.tile([128, OC], f32, tag="ot")
                nc.vector.tensor_scalar_mul(ot[0:cm, :], ps[0:cm, :], 1.0)
                ring_a.dma_start(out_d[o0 : o0 + cm, :], ot[0:cm, :])

    nc.compile()
    return nc


def kernel(x, weights, biases, mode_idx):
    global LAST_EXEC_TIME_NS

    x = np.asarray(x, dtype=np.float32)
    weights = np.asarray(weights, dtype=np.float32)
    biases = np.asarray(biases, dtype=np.float32)
    mode_idx_np = np.asarray(mode_idx).astype(np.int64)

    assert x.shape == (B, I) and weights.shape == (M, O, I)
    assert biases.shape == (M, O) and mode_idx_np.shape == (B,)

    order = np.argsort(mode_idx_np, kind="stable")
    counts = np.bincount(mode_idx_np, minlength=M)
    offs = np.concatenate([[0], np.cumsum(counts)]).astype(int)
    key = (tuple(int(c) for c in counts), NF8)

    if key not in _CACHE:
        _CACHE[key] = _build(key[0], NF8)
    nc = _CACHE[key]

    xs = x[order]                                    # [B, I] sorted by mode
    # fold the fp8 chunks' 2^-12 scale into their x chunks (exact)
    xs_sc = xs.copy()
    if NF8:
        xs_sc[:, 0 : NF8 * 128] *= np.float32(1.0 / F8_SCALE)
    X = np.ascontiguousarray(
        xs_sc.reshape(B, KC, 128).transpose(2, 1, 0).astype(BF16)
    )                                                # [p, k, s]

    # weights [m, o, i] -> [core, m, p(=i%128), k, oc] with per-chunk format
    wr = weights.reshape(M, NCORES, OC, KC, 128)
    in_maps = [dict() for _ in range(NCORES)]
    if NF8:
        W8 = np.ascontiguousarray(
            (wr[:, :, :, :NF8] * np.float32(F8_SCALE))
            .astype(F8)
            .transpose(1, 0, 4, 3, 2)
        ).reshape(NCORES, M, 128, NF8 * OC)
        for c in range(NCORES):
            in_maps[c]["w8"] = W8[c]
    if KC - NF8:
        WB = np.ascontiguousarray(
            wr[:, :, :, NF8:].astype(BF16).transpose(1, 0, 4, 3, 2)
        ).reshape(NCORES, M, 128, (KC - NF8) * OC)
        for c in range(NCORES):
            in_maps[c]["wb"] = WB[c]

    BH = np.ascontiguousarray(
        biases.astype(BF16).reshape(M, NCORES, OC).transpose(1, 0, 2)
    ).reshape(NCORES, 1, M * OC)
    for c in range(NCORES):
        in_maps[c]["x"] = X
        in_maps[c]["b"] = BH[c]

    from concourse.bass_utils import run_bass_kernel_spmd

    trace = bool(int(os.environ.get("BASS_KERNEL_TRACE", "0")))
    if trace:
        _install_ntff_shim()
    res = run_bass_kernel_spmd(
        nc,
        in_maps,
        list(range(NCORES)),
        trace=trace,
        trace_cores=list(range(NCORES)) if trace else None,
    )
    LAST_EXEC_TIME_NS = res.exec_time_ns

    sorted_out = np.concatenate(
        [res.results[c]["out"] for c in range(NCORES)], axis=1
    )
    out = np.empty((B, O), dtype=np.float32)
    out[order] = sorted_out
    return out


# revision 17
# speedup vs baseline: 1.0169x; 1.0169x over previous
"""Trainium2 Bass kernel for BayesLinearEMP (moe_routing).

out[b] = weights[mode_idx[b]] @ x[b] + biases[mode_idx[b]]
  x: [128, 2048] f32, weights: [20, 2048, 2048] f32, biases: [20, 2048] f32,
  mode_idx: [128] int

Strategy (8 NeuronCores):
  - Split the output dim O=2048 into 8 slices of 256, one per core.  Every
    core reads all 20 modes' weights for its O-slice - perfectly balanced
    regardless of the mode distribution, and total weight traffic is
    read-once (the memory-roofline minimum).
  - On the host, sort samples by mode.  Per mode m with count c_m the core
    computes a [c_m, 256] tile as 16 K-chunk matmuls (K=128, N=256) with the
    x chunk as the stationary operand, accumulated in one PSUM group;
    per-mode counts are compile-time constants (program cached per
    counts-tuple).
  - The problem is DMA-bound (weights dominate traffic; streams measured at
    ~340 GB/s per core vs the ~358 GB/s HBM-per-NC limit), so the weight
    encoding is chosen for minimum bytes within the 2e-2 error budget:
    NF8 of the 16 K-chunks are stored as e4m3 fp8 (x1 byte), the rest as
    bf16 (x2 bytes).  The fp8 chunks' 2^12 quantization scale is folded
    into their x chunks (exact power-of-2) so every chunk accumulates into
    the same PSUM group with no extra instructions.
      NF8=0:  21 MB/core,   measured rel err 2.06e-3, ~86 us
      NF8=8:  16.1 MB/core, measured rel err 1.6543e-2, ~69 us
      NF8=9:  15.4 MB/core, measured rel err 1.7566e-2, ~68 us  (default)
    HW reproduces the numpy quantization sim to ~1e-4 relative; the error
    is a norm over 262k outputs, so it concentrates hard even if the
    harness regenerates inputs - no tail risk at 12% margin.
  - DMA plan: x halves lead the two HWDGE rings, then per mode (processed
    largest-count first) the fp8 tile rides one ring and the bf16 tile the
    other, alternating per mode; 12-deep weight pool keeps ~5 modes of
    prefetch in flight so the stream never starves; the last two modes'
    bf16 tiles are split in half so the tail compute overlaps the final
    bytes.  SWDGE (gpsimd) is avoided: its end-of-kernel Q7 drain costs
    ~4 us.
  - The bias is folded into the PSUM accumulation with a K=1 ones-matmul
    against the bf16 bias row; per-mode results are DVE-copied to SBUF and
    written back per mode so output writes overlap the stream.
"""

import os
import sys

for _p in ("/opt/trn_rl_repo", "/root/.axon_site/_ro/trn_rl_repo"):
    if _p not in sys.path:
        sys.path.append(_p)

import numpy as np
import ml_dtypes

BF16 = ml_dtypes.bfloat16
F8 = ml_dtypes.float8_e4m3

B, I, O, M = 128, 2048, 2048, 20
NCORES = 8
OC = O // NCORES          # 256 output cols per core
KC = I // 128             # 16 contraction chunks

NF8 = int(os.environ.get("BASS_NF8", "9"))   # fp8 chunks per 16
F8_SCALE = 4096.0                            # 2^12, folded into x chunks

_CACHE: dict = {}
LAST_EXEC_TIME_NS = None


def _install_ntff_shim():
    """antenv.axon_hooks is absent in this image; recreate it so the
    trace=True path of run_bass_kernel_spmd can reach NTFF profiling."""
    import types
    import antenv

    if getattr(antenv, "axon_hooks", None) is not None:
        return
    hooks_mod = types.ModuleType("antenv.axon_hooks")
    _hook = [None]
    hooks_mod.set_axon_ntff_profile_hook = lambda h: _hook.__setitem__(0, h)
    hooks_mod.get_axon_ntff_profile_hook = lambda: _hook[0]
    sys.modules["antenv.axon_hooks"] = hooks_mod
    antenv.axon_hooks = hooks_mod
    try:
        from trn_agent_boot.trn_boot import _ntff_profile_via_ctypes

        hooks_mod.set_axon_ntff_profile_hook(
            _ntff_profile_via_ctypes("/opt/axon/libaxon_pjrt.so")
        )
    except Exception:
        pass
    import concourse.bass_utils as bass_utils

    bass_utils.upload_artifacts = lambda tmpdir: "local://" + tmpdir


def _build(counts: tuple, nf8: int):
    import concourse.bass as bass
    import concourse.tile as tile
    from concourse import bacc, mybir

    offs = np.concatenate([[0], np.cumsum(counts)]).astype(int)

    nc = bacc.Bacc("TRN2", target_bir_lowering=False, debug=False, num_devices=NCORES)
    bf = mybir.dt.bfloat16
    f8 = mybir.dt.float8e4
    f32 = mybir.dt.float32
    u8 = mybir.dt.uint8

    nbf = KC - nf8
    a_bytes = nf8 * OC            # fp8 chunk bytes per partition
    b_bytes = nbf * OC * 2        # bf16 chunk bytes per partition
    # one merged byte payload per (mode, core): fp8 chunks then bf16 chunks
    wmix_d = nc.dram_tensor(
        "wmix", [M, 128, a_bytes + b_bytes], u8, kind="ExternalInput"
    ).ap()
    x_d = nc.dram_tensor("x", [128, KC, 128], bf, kind="ExternalInput").ap()
    b_d = nc.dram_tensor("b", [1, M * OC], bf, kind="ExternalInput").ap()
    out_d = nc.dram_tensor("out", [B, OC], f32, kind="ExternalOutput").ap()

    # process modes largest-count first: big output tiles drain early and
    # the tail mode is small
    morder = [m for m in np.argsort(-np.asarray(counts), kind="stable") if counts[m]]

    with tile.TileContext(nc) as tc:
        with (
            tc.tile_pool(name="w", bufs=14) as wpool,
            tc.tile_pool(name="x", bufs=1) as xpool,
            tc.tile_pool(name="consts", bufs=1) as cpool,
            tc.tile_pool(name="o", bufs=4) as opool,
            tc.tile_pool(name="ps", bufs=6, space=bass.MemorySpace.PSUM) as pspool,
        ):
            # x leads the sync ring, bias the scalar ring; mode 0's merged W
            # follows the tiny bias on scalar so both rings stream at once
            xt = xpool.tile([128, KC, 128], bf, tag="xt")
            nc.sync.dma_start(xt[:], x_d[:])
            bt = cpool.tile([1, M * OC], bf)
            nc.scalar.dma_start(bt[:], b_d[:])
            ones = cpool.tile([1, 128], bf)
            nc.vector.memset(ones[:], 1.0)

            for mi_pos, m in enumerate(morder):
                cm = int(counts[m])
                o0 = int(offs[m])
                # alternate rings per mode to balance the two HWDGE queues
                ring_a = nc.sync if (mi_pos % 2) else nc.scalar
                ring_b = nc.scalar if (mi_pos % 2) else nc.sync
                # tail modes: two parallel part-DMAs so their compute
                # pipelines with the last bytes of the stream
                split = mi_pos >= len(morder) - 2 and nf8 and nbf
                if split:
                    wAt = wpool.tile([128, a_bytes], u8, tag="wA", bufs=2)
                    ring_a.dma_start(wAt[:], wmix_d[m, :, 0:a_bytes])
                    wBt = wpool.tile([128, b_bytes], u8, tag="wB", bufs=2)
                    ring_b.dma_start(wBt[:], wmix_d[m, :, a_bytes:])
                    w8v = wAt[:].bitcast(f8) if nf8 else None
                    wbv = wBt[:].bitcast(bf) if nbf else None
                else:
                    wt = wpool.tile([128, a_bytes + b_bytes], u8, tag="w")
                    ring_a.dma_start(wt[:], wmix_d[m])
                    w8v = wt[:, 0:a_bytes].bitcast(f8) if nf8 else None
                    wbv = wt[:, a_bytes:].bitcast(bf) if nbf else None

                ps = pspool.tile([128, OC], f32, tag="ps")
                for k in range(KC):
                    if k < nf8:
                        w_k = w8v[:, k * OC : (k + 1) * OC]
                    else:
                        kb = k - nf8
                        w_k = wbv[:, kb * OC : (kb + 1) * OC]
                    x_k = xt[:, k, o0 : o0 + cm]
                    nc.tensor.matmul(
                        ps[0:cm, :], x_k, w_k, start=(k == 0), stop=False
                    )
                # bias: ones[1,cm].T @ b_row
                nc.tensor.matmul(
                    ps[0:cm, :],
                    ones[:, 0:cm],
                    bt[:, m * OC : (m + 1) * OC],
                    start=False,
                    stop=True,
                )
                ot = opool.tile([128, OC], f32, tag="ot")
                nc.vector.tensor_scalar_mul(ot[0:cm, :], ps[0:cm, :], 1.0)
                # output writes ride SWDGE so they never occupy the W rings
                nc.gpsimd.dma_start(out_d[o0 : o0 + cm, :], ot[0:cm, :])

    nc.compile()
    return nc


def kernel(x, weights, biases, mode_idx):
    global LAST_EXEC_TIME_NS

    x = np.asarray(x, dtype=np.float32)
    weights = np.asarray(weights, dtype=np.float32)
    biases = np.asarray(biases, dtype=np.float32)
    mode_idx_np = np.asarray(mode_idx).astype(np.int64)

    assert x.shape == (B, I) and weights.shape == (M, O, I)
    assert biases.shape == (M, O) and mode_idx_np.shape == (B,)

    order = np.argsort(mode_idx_np, kind="stable")
    counts = np.bincount(mode_idx_np, minlength=M)
    offs = np.concatenate([[0], np.cumsum(counts)]).astype(int)
    key = (tuple(int(c) for c in counts), NF8)

    if key not in _CACHE:
        _CACHE[key] = _build(key[0], NF8)
    nc = _CACHE[key]

    xs = x[order]                                    # [B, I] sorted by mode
    # fold the fp8 chunks' 2^-12 scale into their x chunks (exact)
    xs_sc = xs.copy()
    if NF8:
        xs_sc[:, 0 : NF8 * 128] *= np.float32(1.0 / F8_SCALE)
    X = np.ascontiguousarray(
        xs_sc.reshape(B, KC, 128).transpose(2, 1, 0).astype(BF16)
    )                                                # [p, k, s]

    # weights [m, o, i] -> [core, m, p(=i%128), k, oc] with per-chunk format,
    # then the fp8 and bf16 chunk bytes concatenated into one uint8 payload
    wr = weights.reshape(M, NCORES, OC, KC, 128)
    in_maps = [dict() for _ in range(NCORES)]
    parts = []
    if NF8:
        W8 = np.ascontiguousarray(
            (wr[:, :, :, :NF8] * np.float32(F8_SCALE))
            .astype(F8)
            .transpose(1, 0, 4, 3, 2)
        ).reshape(NCORES, M, 128, NF8 * OC)
        parts.append(W8.view(np.uint8))
    if KC - NF8:
        WB = np.ascontiguousarray(
            wr[:, :, :, NF8:].astype(BF16).transpose(1, 0, 4, 3, 2)
        ).reshape(NCORES, M, 128, (KC - NF8) * OC)
        parts.append(WB.view(np.uint8))
    WMIX = np.concatenate(parts, axis=-1)
    for c in range(NCORES):
        in_maps[c]["wmix"] = WMIX[c]

    BH = np.ascontiguousarray(
        biases.astype(BF16).reshape(M, NCORES, OC).transpose(1, 0, 2)
    ).reshape(NCORES, 1, M * OC)
    for c in range(NCORES):
        in_maps[c]["x"] = X
        in_maps[c]["b"] = BH[c]

    from concourse.bass_utils import run_bass_kernel_spmd

    trace = bool(int(os.environ.get("BASS_KERNEL_TRACE", "0")))
    if trace:
        _install_ntff_shim()
    res = run_bass_kernel_spmd(
        nc,
        in_maps,
        list(range(NCORES)),
        trace=trace,
        trace_cores=list(range(NCORES)) if trace else None,
    )
    LAST_EXEC_TIME_NS = res.exec_time_ns

    sorted_out = np.concatenate(
        [res.results[c]["out"] for c in range(NCORES)], axis=1
    )
    out = np.empty((B, O), dtype=np.float32)
    out[order] = sorted_out
    return out
